# revision 1
# baseline (speedup 1.0000x reference)
"""Trainium2 Bass kernel for ModalityAwareDualAttention (dense_cnn).

Sharding: pure data-parallel over batch (32 -> 4 per core x 8 cores).
Per core: loop over P=3 parts; per part process 2 pairs of local batches.

Algebraic restructurings (exact up to fp assoc.):
  - depthwise scale/bias + 2x2-avg-pool 0.25 factor folded into Wq/Wk + biases
  - v computed transposed (vT = xd^T @ Wv^T) so no on-chip weight transpose
  - v-bias commutes through softmax (rows sum to 1); folded into upsample bias
    and (for the SE input) into fc1's bias
  - attention apply + bilinear 2x upsample + pa_gamma fused into two matmuls:
    up_s = vT.T @ (attn_n @ KT), KT = gamma * K_bilinear^T plus an extra
    column of column-means that yields mean(up) for the SE global-avg-pool
  - SE sigmoid gate + modality gate fused into per-channel affine:
    final = xp*(1 + mwc*cw) + up_s*(mw + mwc*cw),  mwc = mw*ca_gamma
Matmuls in bf16 (fp32 PSUM accumulation); residual path kept fp32.
"""

import numpy as np
import ml_dtypes

import concourse.bass as bass
import concourse.tile as tile
import concourse.mybir as mybir

F32 = mybir.dt.float32
BF16 = mybir.dt.bfloat16
AF = mybir.ActivationFunctionType
ALU = mybir.AluOpType

N_CORES = 8
B, C, H, W, P = 32, 2048, 48, 24, 3
BL = B // N_CORES          # 4 local batches per core
IC = 128                   # q/k inter channels
C4 = 512                   # SE bottleneck
PH = H // P                # 16
HD, WD = PH // 2, W // 2   # 8, 12
N = HD * WD                # 96 attention tokens
HWP = PH * W               # 384 spatial positions per part
KC = C // 128              # 16 channel tiles
NPAIR = 2 * N              # 192


def _up_matrix(n):
    """[2n, n] bilinear x2 upsample (align_corners=False, edge clamp)."""
    M = np.zeros((2 * n, n), np.float64)
    for o in range(2 * n):
        src = (o + 0.5) / 2.0 - 0.5
        i0 = int(np.floor(src))
        f = src - i0
        M[o, min(max(i0, 0), n - 1)] += 1.0 - f
        M[o, min(max(i0 + 1, 0), n - 1)] += f
    return M


def k_bilinear():
    """[384, 96] upsample matrix: flat(16,24) <- flat(8,12)."""
    return np.kron(_up_matrix(HD), _up_matrix(WD))


def split_excess_waits(nc, max_waits=1):
    """This walrus build rejects multi-sem-wait instructions on some opcodes;
    hoist extra waits onto preceding same-engine no-ops."""
    for f in nc.m.functions:
        for bb in f.blocks:
            insts = bb.instructions
            i = 0
            while i < len(insts):
                ins = insts[i]
                si = ins.sync_info
                if si is not None and si.on_wait and len(si.on_wait) > max_waits:
                    waits = list(si.on_wait)
                    extra, keep = waits[:-max_waits], waits[-max_waits:]
                    nops = []
                    for s in range(0, len(extra), max_waits):
                        nops.append(mybir.InstNoOp(
                            name=nc.get_next_instruction_name(),
                            engine=ins.engine, ins=[], outs=[],
                            sync_info=mybir.SyncInfo(
                                on_wait=extra[s:s + max_waits], on_update=[]),
                        ))
                    ins.sync_info = mybir.SyncInfo(
                        on_wait=keep, on_update=list(si.on_update or []))
                    insts[i:i] = nops
                    i += len(nops)
                i += 1


def build_program(split_waits=True):
    from contextlib import ExitStack
    nc = bass.Bass()

    x = nc.dram_tensor("x", [BL, C, H, W], F32, kind="ExternalInput")
    wqT = nc.dram_tensor("wqT", [P, C, IC], BF16, kind="ExternalInput")
    wkT = nc.dram_tensor("wkT", [P, C, IC], BF16, kind="ExternalInput")
    qb = nc.dram_tensor("qb", [P, IC], F32, kind="ExternalInput")
    kb = nc.dram_tensor("kb", [P, IC], F32, kind="ExternalInput")
    wvT = nc.dram_tensor("wvT", [P, C, C], BF16, kind="ExternalInput")
    vbg = nc.dram_tensor("vbg", [P, C], F32, kind="ExternalInput")
    ktd = nc.dram_tensor("ktd", [P, N, HWP + 1], BF16, kind="ExternalInput")
    fc1T = nc.dram_tensor("fc1T", [P, C, C4], BF16, kind="ExternalInput")
    b1 = nc.dram_tensor("b1", [P, C4], F32, kind="ExternalInput")
    fc2T = nc.dram_tensor("fc2T", [P, C4, C], BF16, kind="ExternalInput")
    b2 = nc.dram_tensor("b2", [P, C], F32, kind="ExternalInput")
    # gmw[p, 0] = mw*ca_gamma (mwc), gmw[p, 1] = mw; replicated to 128 rows
    gmw = nc.dram_tensor("gmw", [P, 2, 128, BL], F32, kind="ExternalInput")
    out = nc.dram_tensor("out", [BL, C, H, W], F32, kind="ExternalOutput")

    xv = x.ap().rearrange("b c (p h) w -> b c p h w", p=P)
    ov = out.ap().rearrange("b c (p h) w -> b c p h w", p=P)

    with ExitStack() as ctx:
        tc = ctx.enter_context(tile.TileContext(nc))
        pool = lambda name, bufs, **kw: ctx.enter_context(
            tc.tile_pool(name=name, bufs=bufs, **kw))
        wv_pool = pool("wv", KC)
        wqk_pool = pool("wqk", 2 * KC)
        fc1_pool = pool("fc1", 4)
        fc2_pool = pool("fc2", 4)
        kt_pool = pool("ktp", 2)
        xp_pool = pool("xp", 32)
        xd_pool = pool("xd", KC)
        t1_pool = pool("t1", 2)
        qk_pool = pool("qk", 3)
        attn_pool = pool("attn", 4)
        g_pool = pool("gg", 2)
        vt_pool = pool("vt", 2)
        ups_pool = pool("ups", 33)
        sm_pool = pool("sm", 42)
        xs_pool = pool("xs", 40)
        gap_pool = pool("gap", 18)
        se_pool = pool("se", 44)
        fin_pool = pool("fin", 4)
        ps_vt = pool("ps_vt", 1, space="PSUM")
        ps_bank = pool("ps_bank", 4, space="PSUM")
        if True:
            for p in range(P):
                # ---------- per-part weight loads ----------
                wv_t = []
                for kc in range(KC):
                    t = wv_pool.tile([128, C], BF16, tag="wv")
                    nc.sync.dma_start(t[:], wvT.ap()[p, kc * 128:(kc + 1) * 128, :])
                    wv_t.append(t)
                wq_t, wk_t = [], []
                for kc in range(KC):
                    t = wqk_pool.tile([128, IC], BF16, tag="wqk")
                    nc.sync.dma_start(t[:], wqT.ap()[p, kc * 128:(kc + 1) * 128, :])
                    wq_t.append(t)
                    t = wqk_pool.tile([128, IC], BF16, tag="wqk")
                    nc.sync.dma_start(t[:], wkT.ap()[p, kc * 128:(kc + 1) * 128, :])
                    wk_t.append(t)
                kt_t = kt_pool.tile([N, HWP + 1], BF16, tag="kt")
                nc.sync.dma_start(kt_t[:], ktd.ap()[p])
                qb_t = sm_pool.tile([IC, 1], F32, tag="sm")
                nc.sync.dma_start(qb_t[:], qb.ap()[p].unsqueeze(1))
                kb_t = sm_pool.tile([IC, 1], F32, tag="sm")
                nc.sync.dma_start(kb_t[:], kb.ap()[p].unsqueeze(1))
                vbg_t, b2_t = [], []
                for kc in range(KC):
                    t = sm_pool.tile([128, 1], F32, tag="sm")
                    nc.sync.dma_start(
                        t[:], vbg.ap()[p, kc * 128:(kc + 1) * 128].unsqueeze(1))
                    vbg_t.append(t)
                    t = sm_pool.tile([128, 1], F32, tag="sm")
                    nc.sync.dma_start(
                        t[:], b2.ap()[p, kc * 128:(kc + 1) * 128].unsqueeze(1))
                    b2_t.append(t)
                b1_t = []
                for m in range(C4 // 128):
                    t = sm_pool.tile([128, 1], F32, tag="sm")
                    nc.sync.dma_start(
                        t[:], b1.ap()[p, m * 128:(m + 1) * 128].unsqueeze(1))
                    b1_t.append(t)
                mwc_t = sm_pool.tile([128, BL], F32, tag="sm")
                nc.sync.dma_start(mwc_t[:], gmw.ap()[p, 0])
                mw_t = sm_pool.tile([128, BL], F32, tag="sm")
                nc.sync.dma_start(mw_t[:], gmw.ap()[p, 1])

                for pr in range(BL // 2):
                    bs = [2 * pr, 2 * pr + 1]
                    # ---------- load xp, 2x2-sum-pool -> xd (+ row sums) ----------
                    xp_t, xs_t, up_t, gap2 = {}, {}, {}, []
                    xd_t = []
                    for kc in range(KC):
                        xdt = xd_pool.tile([128, NPAIR], BF16, tag="xd")
                        xd_t.append(xdt)
                        g2 = gap_pool.tile([128, 2], BF16, tag="gap")
                        gap2.append(g2)
                        for j, b in enumerate(bs):
                            xt = xp_pool.tile([128, PH, W], F32, tag="xp")
                            nc.sync.dma_start(
                                xt[:], xv[b, kc * 128:(kc + 1) * 128, p])
                            xp_t[(kc, b)] = xt
                            t1 = t1_pool.tile([128, HD, W], F32, tag="t1")
                            nc.vector.tensor_tensor(
                                t1[:], xt[:, 0:PH:2, :], xt[:, 1:PH:2, :], ALU.add)
                            xs = xs_pool.tile([128, 1], F32, tag="xs")
                            xs_t[(kc, b)] = xs
                            t1v = t1[:].rearrange("q h (w two) -> q h w two", two=2)
                            nc.vector.scalar_tensor_tensor(
                                xdt[:, j * N:(j + 1) * N].rearrange(
                                    "q (h w) -> q h w", h=HD),
                                t1v[:, :, :, 0], 1.0, t1v[:, :, :, 1],
                                ALU.mult, ALU.add, accum_out=xs[:])

                    # ---------- q/k projections (batched over pair) ----------
                    q_ps = ps_bank.tile([IC, NPAIR], F32, tag="bank")
                    for kc in range(KC):
                        nc.tensor.matmul(q_ps[:], wq_t[kc][:], xd_t[kc][:],
                                         start=(kc == 0), stop=(kc == KC - 1))
                    q_sb = qk_pool.tile([IC, NPAIR], BF16, tag="qk")
                    nc.scalar.activation(q_sb[:], q_ps[:], AF.Identity, bias=qb_t[:])
                    k_ps = ps_bank.tile([IC, NPAIR], F32, tag="bank")
                    for kc in range(KC):
                        nc.tensor.matmul(k_ps[:], wk_t[kc][:], xd_t[kc][:],
                                         start=(kc == 0), stop=(kc == KC - 1))
                    k_sb = qk_pool.tile([IC, NPAIR], BF16, tag="qk")
                    nc.scalar.activation(k_sb[:], k_ps[:], AF.Identity, bias=kb_t[:])

                    for j, b in enumerate(bs):
                        qs = q_sb[:, j * N:(j + 1) * N]
                        ks = k_sb[:, j * N:(j + 1) * N]
                        # ---------- attention (softmax without max-shift:
                        # |energy| ~ 1e-3, exp cannot overflow) ----------
                        e_ps = ps_bank.tile([N, N], F32, tag="bank")
                        nc.tensor.matmul(e_ps[:], qs, ks, start=True, stop=True)
                        attn_e = attn_pool.tile([N, N], BF16, tag="attn")
                        s_sum = xs_pool.tile([N, 1], F32, tag="xs")
                        nc.scalar.activation(attn_e[:], e_ps[:], AF.Exp,
                                             accum_out=s_sum[:])
                        r_sum = xs_pool.tile([N, 1], F32, tag="xs")
                        nc.vector.reciprocal(r_sum[:], s_sum[:])
                        attn_n = attn_pool.tile([N, N], BF16, tag="attn")
                        nc.vector.tensor_scalar(attn_n[:], attn_e[:], r_sum[:],
                                                None, ALU.mult)
                        # ---------- G = attn_n @ KT  [N, 385] ----------
                        g_ps = ps_bank.tile([N, HWP + 1], F32, tag="bank")
                        nc.tensor.matmul(g_ps[:], attn_n[:], kt_t[:],
                                         start=True, stop=True)
                        g_sb = g_pool.tile([N, HWP + 1], BF16, tag="g")
                        nc.scalar.activation(g_sb[:], g_ps[:], AF.Copy)
                        # ---------- vT = xd_b^T @ WvT  [N, C] ----------
                        vt_ps = ps_vt.tile([N, C], F32, tag="vt")
                        for kc in range(KC):
                            xdb = xd_t[kc][:, j * N:(j + 1) * N]
                            for bk in range(4):
                                nc.tensor.matmul(
                                    vt_ps[:, bk * 512:(bk + 1) * 512], xdb,
                                    wv_t[kc][:, bk * 512:(bk + 1) * 512],
                                    start=(kc == 0), stop=(kc == KC - 1))
                        vt_sb = vt_pool.tile([N, C], BF16, tag="vt")
                        for bk in range(4):
                            nc.scalar.activation(
                                vt_sb[:, bk * 512:(bk + 1) * 512],
                                vt_ps[:, bk * 512:(bk + 1) * 512], AF.Copy)
                        # ---------- up_s = vT^T @ G (+ gamma*vb); gap ----------
                        for kc in range(KC):
                            up_ps = ps_bank.tile([128, HWP + 1], F32, tag="bank")
                            nc.tensor.matmul(
                                up_ps[:], vt_sb[:, kc * 128:(kc + 1) * 128],
                                g_sb[:], start=True, stop=True)
                            upt = ups_pool.tile([128, HWP], BF16, tag="ups")
                            nc.scalar.activation(upt[:], up_ps[:, 0:HWP],
                                                 AF.Identity, bias=vbg_t[kc][:])
                            up_t[(kc, b)] = upt
                            # gap (for SE) excludes the gamma*vb term; it is
                            # folded into fc1's bias host-side
                            nc.vector.scalar_tensor_tensor(
                                gap2[kc][:, j:j + 1], xs_t[(kc, b)][:],
                                1.0 / HWP, up_ps[:, HWP:HWP + 1],
                                ALU.mult, ALU.add)

                    # ---------- SE gate (batched over pair, free dim 2) -------
                    h_ps = [ps_bank.tile([128, 2], F32, tag="bank", name=f"hps{m}")
                            for m in range(C4 // 128)]
                    for kc in range(KC):
                        ft = fc1_pool.tile([128, C4], BF16, tag="fc1")
                        nc.sync.dma_start(
                            ft[:], fc1T.ap()[p, kc * 128:(kc + 1) * 128, :])
                        for m in range(C4 // 128):
                            nc.tensor.matmul(
                                h_ps[m][:], ft[:, m * 128:(m + 1) * 128],
                                gap2[kc][:], start=(kc == 0), stop=(kc == KC - 1))
                    h1_t = []
                    for m in range(C4 // 128):
                        hb = se_pool.tile([128, 2], BF16, tag="se")
                        nc.scalar.activation(hb[:], h_ps[m][:], AF.Relu,
                                             bias=b1_t[m][:])
                        h1_t.append(hb)
                    fc2_t = []
                    for m in range(C4 // 128):
                        t = fc2_pool.tile([128, C], BF16, tag="fc2")
                        nc.sync.dma_start(
                            t[:], fc2T.ap()[p, m * 128:(m + 1) * 128, :])
                        fc2_t.append(t)
                    cw12 = []
                    for kc in range(KC):
                        c_ps = ps_bank.tile([128, 2], F32, tag="bank")
                        for m in range(C4 // 128):
                            nc.tensor.matmul(
                                c_ps[:], fc2_t[m][:, kc * 128:(kc + 1) * 128],
                                h1_t[m][:], start=(m == 0), stop=(m == 3))
                        cw = se_pool.tile([128, 2], F32, tag="se")
                        nc.scalar.activation(cw[:], c_ps[:], AF.Sigmoid,
                                             bias=b2_t[kc][:])
                        tmp = se_pool.tile([128, 2], F32, tag="se")
                        nc.vector.tensor_tensor(
                            tmp[:], cw[:], mwc_t[:, 2 * pr:2 * pr + 2], ALU.mult)
                        cw1 = se_pool.tile([128, 2], F32, tag="se")
                        nc.vector.tensor_scalar(cw1[:], tmp[:], 1.0, None, ALU.add)
                        cw2 = se_pool.tile([128, 2], F32, tag="se")
                        nc.vector.tensor_tensor(
                            cw2[:], tmp[:], mw_t[:, 2 * pr:2 * pr + 2], ALU.add)
                        cw12.append((cw1, cw2))

                    # ---------- final blend + store ----------
                    for j, b in enumerate(bs):
                        for kc in range(KC):
                            cw1, cw2 = cw12[kc]
                            xt = xp_t[(kc, b)]
                            r1 = fin_pool.tile([128, HWP], F32, tag="fin")
                            nc.scalar.activation(
                                r1[:], xt[:].rearrange("q h w -> q (h w)"),
                                AF.Copy, scale=cw1[:, j:j + 1])
                            ot = fin_pool.tile([128, HWP], F32, tag="fin")
                            nc.vector.scalar_tensor_tensor(
                                ot[:], up_t[(kc, b)][:], cw2[:, j:j + 1], r1[:],
                                ALU.mult, ALU.add)
                            nc.sync.dma_start(
                                ov[b, kc * 128:(kc + 1) * 128, p],
                                ot[:].rearrange("q (h w) -> q h w", h=PH))

    if split_waits:
        split_excess_waits(nc)
    return nc


# ---------------------------------------------------------------------------
# Host side
# ---------------------------------------------------------------------------

def _sigmoid(v):
    return 1.0 / (1.0 + np.exp(-v))


def _bf(a):
    return np.ascontiguousarray(a.astype(ml_dtypes.bfloat16))


def _f32(a):
    return np.ascontiguousarray(np.asarray(a, dtype=np.float32))


def prepare_host_inputs(inputs):
    """Fold/transpose weights; returns dict of shared (per-core-identical)
    arrays plus the per-core batch shards of x."""
    g = {k: np.asarray(v) for k, v in inputs.items()}
    x = _f32(g["x"])

    # modality gate on host (tiny): mw [B, P]
    mf = g["modality"].astype(np.float64)[:, None]
    g1 = np.maximum(mf @ g["gate_w1"].astype(np.float64).T
                    + g["gate_b1"].astype(np.float64), 0.0)
    mw = _sigmoid(g1 @ g["gate_w2"].astype(np.float64).T
                  + g["gate_b2"].astype(np.float64))      # [B, P]

    paq = g["pa_q_w"].astype(np.float64)    # [P, IC, C]
    pak = g["pa_k_w"].astype(np.float64)
    pav = g["pa_v_w"].astype(np.float64)    # [P, C, C]
    dwq_w = g["pa_dw_q_w"].astype(np.float64)   # [P, C]
    dwq_b = g["pa_dw_q_b"].astype(np.float64)
    dwk_w = g["pa_dw_k_w"].astype(np.float64)
    dwk_b = g["pa_dw_k_b"].astype(np.float64)
    gam = g["pa_gamma"].astype(np.float64)      # [P]
    cgam = g["ca_gamma"].astype(np.float64)

    wqT = np.stack([(paq[p] * dwq_w[p][None, :] * 0.25).T for p in range(P)])
    wkT = np.stack([(pak[p] * dwk_w[p][None, :] * 0.25).T for p in range(P)])
    qb = np.stack([g["pa_q_b"][p] + paq[p] @ dwq_b[p] for p in range(P)])
    kb = np.stack([g["pa_k_b"][p] + pak[p] @ dwk_b[p] for p in range(P)])
    wvT = np.stack([0.25 * pav[p].T for p in range(P)])
    vbg = np.stack([gam[p] * g["pa_v_b"][p] for p in range(P)])   # [P, C]

    kb_mat = k_bilinear()                     # [384, 96]
    ktd = np.stack([
        gam[p] * np.concatenate(
            [kb_mat.T, kb_mat.mean(axis=0)[:, None]], axis=1)  # [96, 385]
        for p in range(P)])

    fc1 = g["ca_fc1_w"].astype(np.float64)    # [P, C4, C]
    fc2 = g["ca_fc2_w"].astype(np.float64)    # [P, C, C4]
    fc1T = np.stack([fc1[p].T for p in range(P)])
    fc2T = np.stack([fc2[p].T for p in range(P)])
    b1 = np.stack([g["ca_fc1_b"][p] + fc1[p] @ (gam[p] * g["pa_v_b"][p].astype(np.float64))
                   for p in range(P)])
    b2 = _f32(g["ca_fc2_b"])

    shared = {
        "wqT": _bf(wqT), "wkT": _bf(wkT), "qb": _f32(qb), "kb": _f32(kb),
        "wvT": _bf(wvT), "vbg": _f32(vbg), "ktd": _bf(ktd),
        "fc1T": _bf(fc1T), "b1": _f32(b1), "fc2T": _bf(fc2T), "b2": b2,
    }
    # per-core gmw [P, 2, 128, BL]
    per_core = []
    for c in range(N_CORES):
        mwl = mw[c * BL:(c + 1) * BL]         # [BL, P]
        gmw = np.empty((P, 2, 128, BL), np.float32)
        for p in range(P):
            gmw[p, 0, :, :] = (mwl[:, p] * cgam[p])[None, :]
            gmw[p, 1, :, :] = mwl[:, p][None, :]
        per_core.append({
            "x": np.ascontiguousarray(x[c * BL:(c + 1) * BL]),
            "gmw": gmw,
            **shared,
        })
    return per_core


_CACHE = {}


def kernel(**inputs):
    from concourse.bass_utils import run_bass_kernel_spmd

    per_core = prepare_host_inputs(inputs)
    if "nc" not in _CACHE:
        _CACHE["nc"] = build_program()
    nc = _CACHE["nc"]
    res = run_bass_kernel_spmd(nc, per_core, list(range(N_CORES)))
    outs = [res.results[c]["out"] for c in range(N_CORES)]
    return np.concatenate(outs, axis=0).astype(np.float32)



# revision 6
# speedup vs baseline: 1.3867x; 1.3867x over previous
"""Trainium2 Bass kernel for ModalityAwareDualAttention (dense_cnn).

Sharding: pure data-parallel over batch (32 -> 4 per core x 8 cores).
Per core: loop over P=3 parts; each part processes all BL=4 local batches
together (token-packed attention-value matmuls, batched SE gate).

Algebraic restructurings (exact up to fp assoc.):
  - depthwise scale/bias + 2x2-avg-pool 0.25 factor folded into Wq/Wk + biases
  - v computed transposed (vT = xd^T @ Wv^T), token-packed across the 4
    batches (384 tokens -> 3 full 128-wide PE column groups)
  - v-bias commutes through softmax (rows sum to 1); added at up-eviction
  - attention apply + bilinear 2x upsample + pa_gamma fused into two matmuls:
    up = vT^T @ (attn_n @ KT), KT = gamma * K_bilinear^T
  - s := xp + up_sb (pa_out) stored at eviction; its accum gives the SE
    global-avg-pool for free (1/384 folded into fc1 weights)
  - gates folded:  cw2 = mw + mw*ca_gamma*cw,  and since cw1 - cw2 = 1 - mw,
    final = cw2*s + (1-mw)*xp  -- one fused op per tile at blend time
Matmuls and main streams bf16 (fp32 PSUM accumulation); x and out are
carried bf16 end-to-end (tolerance 2e-2 >> bf16 rounding).
"""

import numpy as np
import ml_dtypes

import concourse.bass as bass
import concourse.tile as tile
import concourse.mybir as mybir

F32 = mybir.dt.float32
BF16 = mybir.dt.bfloat16
AF = mybir.ActivationFunctionType
ALU = mybir.AluOpType

N_CORES = 8
B, C, H, W, P = 32, 2048, 48, 24, 3
BL = B // N_CORES          # 4 local batches per core
IC = 128                   # q/k inter channels
C4 = 512                   # SE bottleneck
PH = H // P                # 16
HD, WD = PH // 2, W // 2   # 8, 12
N = HD * WD                # 96 attention tokens per batch
HWP = PH * W               # 384 spatial positions per part
KC = C // 128              # 16 channel tiles
TOK = BL * N               # 384 packed tokens per part
NG = TOK // 128            # 3 token M-groups

# smalls column map
QB, KB_, B1, VBG, B2, MWC, MW, MWM1 = 0, 1, 2, 6, 22, 38, 42, 46
NS = 50


def _up_matrix(n):
    """[2n, n] bilinear x2 upsample (align_corners=False, edge clamp)."""
    M = np.zeros((2 * n, n), np.float64)
    for o in range(2 * n):
        src = (o + 0.5) / 2.0 - 0.5
        i0 = int(np.floor(src))
        f = src - i0
        M[o, min(max(i0, 0), n - 1)] += 1.0 - f
        M[o, min(max(i0 + 1, 0), n - 1)] += f
    return M


def k_bilinear():
    """[384, 96] upsample matrix: flat(16,24) <- flat(8,12)."""
    return np.kron(_up_matrix(HD), _up_matrix(WD))


def split_excess_waits(nc, max_waits=1):
    """This walrus build rejects multi-sem-wait instructions on some opcodes;
    hoist extra waits onto preceding same-engine no-ops."""
    for f in nc.m.functions:
        for bb in f.blocks:
            insts = bb.instructions
            i = 0
            while i < len(insts):
                ins = insts[i]
                si = ins.sync_info
                if si is not None and si.on_wait and len(si.on_wait) > max_waits:
                    waits = list(si.on_wait)
                    extra, keep = waits[:-max_waits], waits[-max_waits:]
                    nops = []
                    for s in range(0, len(extra), max_waits):
                        nops.append(mybir.InstNoOp(
                            name=nc.get_next_instruction_name(),
                            engine=ins.engine, ins=[], outs=[],
                            sync_info=mybir.SyncInfo(
                                on_wait=extra[s:s + max_waits], on_update=[]),
                        ))
                    ins.sync_info = mybir.SyncInfo(
                        on_wait=keep, on_update=list(si.on_update or []))
                    insts[i:i] = nops
                    i += len(nops)
                i += 1


def build_program(split_waits=True):
    from contextlib import ExitStack
    nc = bass.Bass()

    x = nc.dram_tensor("x", [BL, C, H, W], BF16, kind="ExternalInput")
    wq = nc.dram_tensor("wq", [P, 128, KC * IC], BF16, kind="ExternalInput")
    wk = nc.dram_tensor("wk", [P, 128, KC * IC], BF16, kind="ExternalInput")
    wv = nc.dram_tensor("wv", [P, 128, KC, C], BF16, kind="ExternalInput")
    kt = nc.dram_tensor("kt", [P, N, HWP], BF16, kind="ExternalInput")
    fc1 = nc.dram_tensor("fc1", [P, 4, 128, KC * 128], BF16,
                         kind="ExternalInput")
    fc2 = nc.dram_tensor("fc2", [P, 4, 128, 4 * C4], BF16,
                         kind="ExternalInput")
    sm = nc.dram_tensor("sm", [P, 128, NS], F32, kind="ExternalInput")
    out = nc.dram_tensor("out", [BL, C, H, W], BF16, kind="ExternalOutput")

    # x[b, (kc q), (p h), w] -> [b, q, kc, p, (h w)]
    xr = x.ap().rearrange("b (kc q) (p h) w -> b q kc p (h w)",
                          kc=KC, q=128, p=P)
    # out[b, (kg i q), (p h), w] -> [b, kg, q, i, p, (h w)]
    orr = out.ap().rearrange("b (kg i q) (p h) w -> b kg q i p (h w)",
                             kg=8, i=2, q=128, p=P)

    with ExitStack() as ctx:
        tc = ctx.enter_context(tile.TileContext(nc))
        pool = lambda name, bufs, **kw: ctx.enter_context(
            tc.tile_pool(name=name, bufs=bufs, **kw))
        xp_pool = pool("xp", BL + 1)
        xd_pool = pool("xd", KC)
        t1_pool = pool("t1", 4)
        wq_pool = pool("wqp", 2)
        wk_pool = pool("wkp", 2)
        wv_pool = pool("wvp", 3)
        kt_pool = pool("ktp", 2)
        sm_pool = pool("smp", 2)
        fc1_pool = pool("fc1p", 2)
        fc2_pool = pool("fc2p", 2)
        qk_pool = pool("qk", 4)
        at_pool = pool("at", 4)
        ss_pool = pool("ss", 8)
        g_pool = pool("gg", 5)
        vt_pool = pool("vt", NG)
        vtb_pool = pool("vtb", NG)
        s_pool = pool("sp", BL)
        ga_pool = pool("ga", 2)
        gap_pool = pool("gap", 2)
        h1_pool = pool("h1", 8)
        cws_pool = pool("cws", 4)
        cw_pool = pool("cw", 2)
        fin_pool = pool("fin", 3)
        ps_sm = pool("ps_sm", 2, space="PSUM")
        ps_vt = pool("ps_vt", NG, space="PSUM")

        # deferred blend state from previous part
        prev = None

        def emit_blend_batch(st, b):
            """Blend batch b of a finished part: out = cw2*s + (1-mw)*xp."""
            p0, xp0, s0, cw0 = st
            for kg in range(8):
                fin = fin_pool.tile([128, 2, HWP], BF16, tag="fin",
                                    name="fin")
                for i in range(2):
                    kc = kg * 2 + i
                    nc.vector.scalar_tensor_tensor(
                        fin[:, i, :], s0[b][:, kc, :],
                        cw0[:, kc, b:b + 1], xp0[b][:, kc, :],
                        ALU.mult, ALU.add)
                eng = nc.gpsimd if kg % 2 == 0 else nc.sync
                eng.dma_start(orr[b, kg, :, :, p0], fin[:])

        for p in range(P):
            # ---------- per-part weight DMAs ----------
            wq_t = wq_pool.tile([128, KC * IC], BF16, tag="wq", name="wq")
            nc.sync.dma_start(wq_t[:], wq.ap()[p])
            wk_t = wk_pool.tile([128, KC * IC], BF16, tag="wk", name="wk")
            nc.sync.dma_start(wk_t[:], wk.ap()[p])
            kt_t = kt_pool.tile([N, HWP], BF16, tag="kt", name="kt")
            nc.sync.dma_start(kt_t[:], kt.ap()[p])
            sm_t = sm_pool.tile([128, NS], F32, tag="sm", name="sm")
            nc.sync.dma_start(sm_t[:], sm.ap()[p])

            # ---------- interleaved: blend(prev) | load+pool+qk(p) -------
            xp_t, xd_t, qs_t, ks_t = [], [], [], []
            for kc in range(KC):
                xd_t.append(xd_pool.tile([128, TOK], BF16, tag="xd",
                                         name="xd"))
            for b in range(BL):
                if prev is not None:
                    emit_blend_batch(prev, b)
                t = xp_pool.tile([128, KC, HWP], BF16, tag="xp", name="xp")
                nc.sync.dma_start(t[:], xr[b, :, :, p])
                xp_t.append(t)
                for kc in range(KC):
                    xv = xp_t[b][:, kc, :].rearrange(
                        "q (h w) -> q h w", h=PH)
                    t1 = t1_pool.tile([128, HD, W], BF16, tag="t1",
                                      name="t1")
                    eng = nc.vector if (kc % 3 == 0) else nc.gpsimd
                    eng.tensor_tensor(
                        t1[:], xv[:, 0:PH:2, :], xv[:, 1:PH:2, :], ALU.add)
                    t1v = t1[:].rearrange("q h (w two) -> q h w two", two=2)
                    nc.vector.scalar_tensor_tensor(
                        xd_t[kc][:, b * N:(b + 1) * N].rearrange(
                            "q (h w) -> q h w", h=HD),
                        t1v[:, :, :, 0], 1.0, t1v[:, :, :, 1],
                        ALU.mult, ALU.add)
                # q/k per pair once both batches pooled
                if b % 2 == 1:
                    pr = b // 2
                    q_ps = ps_sm.tile([IC, 2 * N], F32, tag="ps", name="qps")
                    for kc in range(KC):
                        nc.tensor.matmul(
                            q_ps[:], wq_t[:, kc * IC:(kc + 1) * IC],
                            xd_t[kc][:, pr * 2 * N:(pr + 1) * 2 * N],
                            start=(kc == 0), stop=(kc == KC - 1))
                    qs = qk_pool.tile([IC, 2 * N], BF16, tag="qk", name="qs")
                    nc.scalar.activation(qs[:], q_ps[:], AF.Identity,
                                         bias=sm_t[:, QB:QB + 1])
                    qs_t.append(qs)
                    k_ps = ps_sm.tile([IC, 2 * N], F32, tag="ps", name="kps")
                    for kc in range(KC):
                        nc.tensor.matmul(
                            k_ps[:], wk_t[:, kc * IC:(kc + 1) * IC],
                            xd_t[kc][:, pr * 2 * N:(pr + 1) * 2 * N],
                            start=(kc == 0), stop=(kc == KC - 1))
                    ks = qk_pool.tile([IC, 2 * N], BF16, tag="qk", name="ks")
                    nc.scalar.activation(ks[:], k_ps[:], AF.Identity,
                                         bias=sm_t[:, KB_:KB_ + 1])
                    ks_t.append(ks)
            # fc1/fc2 streamed chunks (used only at SE time, loaded early)
            fc1_c = []
            for m in range(4):
                t = fc1_pool.tile([128, KC * 128], BF16, tag="fc1",
                                  name="fc1")
                nc.sync.dma_start(t[:], fc1.ap()[p, m])
                fc1_c.append(t)
            fc2_c = []
            for kg in range(4):
                t = fc2_pool.tile([128, 4 * C4], BF16, tag="fc2",
                                  name="fc2")
                nc.sync.dma_start(t[:], fc2.ap()[p, kg])
                fc2_c.append(t)

            # ---------- vT (token-packed, wv streamed in column halves),
            # with per-batch attention interleaved into ch 0 ----------
            vt_t = [vt_pool.tile([128, C], BF16, tag="vt", name="vt")
                    for _ in range(NG)]
            vtb_t = [None] + [vtb_pool.tile([N, C], BF16, tag="vtb",
                                            name="vtb")
                              for _ in range(BL - 1)]
            attn_n = [None] * BL
            g_sb = [None] * BL
            for ch in range(2):
                cl, chw = ch * 1024, 1024
                vt_ps = [ps_vt.tile([128, 1024], F32, tag="psv", name="vps")
                         for _ in range(NG)]
                for kc in range(KC):
                    wv_t = wv_pool.tile([128, 1024], BF16, tag="wv",
                                        name="wv")
                    nc.sync.dma_start(wv_t[:], wv.ap()[p, :, kc,
                                                       cl:cl + chw])
                    for gi in range(NG):
                        for bk in range(2):
                            nc.tensor.matmul(
                                vt_ps[gi][:, bk * 512:(bk + 1) * 512],
                                xd_t[kc][:, gi * 128:(gi + 1) * 128],
                                wv_t[:, bk * 512:(bk + 1) * 512],
                                start=(kc == 0), stop=(kc == KC - 1))
                    # interleave attention (softmax without max-shift:
                    # |energy| ~ 1e-3, exp cannot overflow)
                    if ch == 0 and kc < 2 * BL:
                        j, ph = kc // 2, kc % 2
                        if ph == 0:
                            pr, jo = j // 2, (j % 2) * N
                            e_ps = ps_sm.tile([N, N], F32, tag="ps",
                                              name="eps")
                            nc.tensor.matmul(
                                e_ps[:], qs_t[pr][:, jo:jo + N],
                                ks_t[pr][:, jo:jo + N],
                                start=True, stop=True)
                            a_e = at_pool.tile([N, N], BF16, tag="at",
                                               name="ae")
                            s_sum = ss_pool.tile([N, 1], F32, tag="ss",
                                                 name="ssum")
                            nc.scalar.activation(a_e[:], e_ps[:], AF.Exp,
                                                 accum_out=s_sum[:])
                            r_sum = ss_pool.tile([N, 1], F32, tag="ss",
                                                 name="rsum")
                            nc.vector.reciprocal(r_sum[:], s_sum[:])
                            a_n = at_pool.tile([N, N], BF16, tag="at",
                                               name="an")
                            nc.vector.tensor_scalar(a_n[:], a_e[:],
                                                    r_sum[:], None,
                                                    ALU.mult)
                            attn_n[j] = a_n
                        else:
                            g_ps = ps_sm.tile([N, HWP], F32, tag="ps",
                                              name="gps")
                            nc.tensor.matmul(g_ps[:], attn_n[j][:],
                                             kt_t[:], start=True, stop=True)
                            gt = g_pool.tile([N, HWP], BF16, tag="g",
                                             name="g")
                            nc.scalar.activation(gt[:], g_ps[:], AF.Copy)
                            g_sb[j] = gt
                for gi in range(NG):
                    nc.scalar.activation(vt_t[gi][:, cl:cl + chw],
                                         vt_ps[gi][:], AF.Copy)
                # token re-base: batch j's 96 token rows contiguous from 0
                cs = slice(cl, cl + chw)
                nc.sync.dma_start(vtb_t[1][0:32, cs], vt_t[0][96:128, cs])
                nc.sync.dma_start(vtb_t[1][32:96, cs], vt_t[1][0:64, cs])
                nc.sync.dma_start(vtb_t[2][0:64, cs], vt_t[1][64:128, cs])
                nc.sync.dma_start(vtb_t[2][64:96, cs], vt_t[2][0:32, cs])
                nc.sync.dma_start(vtb_t[3][0:96, cs], vt_t[2][32:128, cs])

            # ---------- up; s = xp + up + gamma*vb; accum -> gap;
            # xp scaled in place by (1-mw) for the final blend ----------
            s_t = [s_pool.tile([128, KC, HWP], BF16, tag="sp", name="s")
                   for _ in range(BL)]
            ga_t = ga_pool.tile([128, KC, BL], F32, tag="ga", name="ga")
            gap_t = gap_pool.tile([128, KC, BL], BF16, tag="gap", name="gap")
            for kc in range(KC):
                for j in range(BL):
                    lhs = (vt_t[0] if j == 0 else vtb_t[j])
                    lhs = lhs[0:N, kc * 128:(kc + 1) * 128]
                    pp = ps_sm if ((kc * BL + j) % 2 == 0) else ps_vt
                    up_ps = pp.tile([128, HWP], F32,
                                    tag="ps" if pp is ps_sm else "psv",
                                    name="ups")
                    nc.tensor.matmul(up_ps[:], lhs, g_sb[j][:],
                                     start=True, stop=True)
                    nc.vector.scalar_tensor_tensor(
                        s_t[j][:, kc, :], up_ps[:],
                        sm_t[:, VBG + kc:VBG + kc + 1], xp_t[j][:, kc, :],
                        ALU.add, ALU.add,
                        accum_out=ga_t[:, kc, j:j + 1])
                    nc.scalar.activation(
                        xp_t[j][:, kc, :], xp_t[j][:, kc, :], AF.Copy,
                        scale=sm_t[:, MWM1 + j:MWM1 + j + 1])
                nc.gpsimd.tensor_copy(gap_t[:, kc, :], ga_t[:, kc, :])

            # ---------- SE gate (batched over 4 batches) ----------
            h1_t = []
            for m in range(C4 // 128):
                h_ps = ps_sm.tile([128, BL], F32, tag="ps", name="hps")
                for kc in range(KC):
                    nc.tensor.matmul(
                        h_ps[:],
                        fc1_c[m][:, kc * 128:(kc + 1) * 128],
                        gap_t[:, kc, :],
                        start=(kc == 0), stop=(kc == KC - 1))
                hb = h1_pool.tile([128, BL], BF16, tag="h1", name="h1")
                nc.scalar.activation(hb[:], h_ps[:], AF.Relu,
                                     bias=sm_t[:, B1 + m:B1 + m + 1])
                h1_t.append(hb)
            cw2_t = cw_pool.tile([128, KC, BL], F32, tag="cw", name="cw2")
            for kc in range(KC):
                c_ps = ps_sm.tile([128, BL], F32, tag="ps", name="cps")
                for m in range(C4 // 128):
                    nc.tensor.matmul(
                        c_ps[:],
                        fc2_c[kc // 4][:, (kc % 4) * C4 + m * 128:
                                       (kc % 4) * C4 + (m + 1) * 128],
                        h1_t[m][:], start=(m == 0), stop=(m == 3))
                cwg = cws_pool.tile([128, BL], F32, tag="cws", name="cwg")
                nc.scalar.activation(cwg[:], c_ps[:], AF.Sigmoid,
                                     bias=sm_t[:, B2 + kc:B2 + kc + 1])
                # cw2 = mw + mwc*cw
                tmp = cws_pool.tile([128, BL], F32, tag="cws", name="tmp")
                nc.vector.tensor_tensor(tmp[:], cwg[:],
                                        sm_t[:, MWC:MWC + BL], ALU.mult)
                nc.vector.tensor_tensor(cw2_t[:, kc, :], tmp[:],
                                        sm_t[:, MW:MW + BL], ALU.add)

            prev = (p, xp_t, s_t, cw2_t)

        # final part's blend
        for b in range(BL):
            emit_blend_batch(prev, b)

    if split_waits:
        split_excess_waits(nc)
    return nc


# ---------------------------------------------------------------------------
# Host side
# ---------------------------------------------------------------------------

def _sigmoid(v):
    return 1.0 / (1.0 + np.exp(-v))


def _bf(a):
    return np.ascontiguousarray(a.astype(ml_dtypes.bfloat16))


def prepare_host_inputs(inputs):
    """Fold/transpose weights; returns per-core input dicts."""
    g = {k: np.asarray(v) for k, v in inputs.items()}
    x = np.asarray(g["x"], dtype=np.float32)

    # modality gate on host (tiny): mw [B, P]
    mf = g["modality"].astype(np.float64)[:, None]
    g1 = np.maximum(mf @ g["gate_w1"].astype(np.float64).T
                    + g["gate_b1"].astype(np.float64), 0.0)
    mw = _sigmoid(g1 @ g["gate_w2"].astype(np.float64).T
                  + g["gate_b2"].astype(np.float64))      # [B, P]

    paq = g["pa_q_w"].astype(np.float64)    # [P, IC, C]
    pak = g["pa_k_w"].astype(np.float64)
    pav = g["pa_v_w"].astype(np.float64)    # [P, C, C]
    dwq_w = g["pa_dw_q_w"].astype(np.float64)   # [P, C]
    dwq_b = g["pa_dw_q_b"].astype(np.float64)
    dwk_w = g["pa_dw_k_w"].astype(np.float64)
    dwk_b = g["pa_dw_k_b"].astype(np.float64)
    gam = g["pa_gamma"].astype(np.float64)      # [P]
    cgam = g["ca_gamma"].astype(np.float64)

    def chunkT(a, n128, inner):
        # [C_outer, inner] -> [128, n128 * inner], partition-major
        return a.reshape(n128, 128, inner).transpose(1, 0, 2).reshape(
            128, n128 * inner)

    wq_h = np.stack([chunkT((paq[pp] * dwq_w[pp][None, :] * 0.25).T, KC, IC)
                     for pp in range(P)])
    wk_h = np.stack([chunkT((pak[pp] * dwk_w[pp][None, :] * 0.25).T, KC, IC)
                     for pp in range(P)])
    qb_h = np.stack([g["pa_q_b"][pp] + paq[pp] @ dwq_b[pp] for pp in range(P)])
    kb_h = np.stack([g["pa_k_b"][pp] + pak[pp] @ dwk_b[pp] for pp in range(P)])
    wv_h = np.stack([chunkT(0.25 * pav[pp].T, KC, C).reshape(128, KC, C)
                     for pp in range(P)])
    vbg_h = np.stack([(gam[pp] * g["pa_v_b"][pp].astype(np.float64))
                      .reshape(KC, 128).T for pp in range(P)])  # [P,128,16]

    kb_mat = k_bilinear()                     # [384, 96]
    kt_h = np.stack([gam[pp] * kb_mat.T for pp in range(P)])    # [P,96,384]

    fc1w = g["ca_fc1_w"].astype(np.float64)    # [P, C4, C]
    fc2w = g["ca_fc2_w"].astype(np.float64)    # [P, C, C4]
    # fc1': (fc1_w/384).T [C, C4] -> [m, q(c chunk part.), kc, j] chunks
    fc1_h = np.stack([
        (fc1w[pp] / HWP).T.reshape(KC, 128, 4, 128)
        .transpose(2, 1, 0, 3).reshape(4, 128, KC * 128)
        for pp in range(P)])
    # fc2': fc2_w.T [C4, C] -> [kcg, q(c4 chunk part.), ci, m, j]
    fc2_h = np.stack([
        fc2w[pp].T.reshape(4, 128, 4, 4, 128)
        .transpose(2, 1, 3, 0, 4).reshape(4, 128, 4 * C4)
        for pp in range(P)])
    b1_h = np.stack([g["ca_fc1_b"][pp].reshape(4, 128).T for pp in range(P)])
    b2_h = np.stack([g["ca_fc2_b"][pp].reshape(KC, 128).T for pp in range(P)])

    shared = {
        "wq": _bf(wq_h), "wk": _bf(wk_h), "wv": _bf(wv_h),
        "kt": _bf(kt_h), "fc1": _bf(fc1_h.reshape(P, 4, 128, KC * 128)),
        "fc2": _bf(fc2_h.reshape(P, 4, 128, 4 * C4)),
    }
    xbf = x.astype(ml_dtypes.bfloat16)
    per_core = []
    for cc in range(N_CORES):
        mwl = mw[cc * BL:(cc + 1) * BL]       # [BL, P]
        sm_h = np.zeros((P, 128, NS), np.float32)
        for pp in range(P):
            sm_h[pp, :, QB] = qb_h[pp]
            sm_h[pp, :, KB_] = kb_h[pp]
            sm_h[pp, :, B1:B1 + 4] = b1_h[pp]
            sm_h[pp, :, VBG:VBG + KC] = vbg_h[pp]
            sm_h[pp, :, B2:B2 + KC] = b2_h[pp]
            sm_h[pp, :, MWC:MWC + BL] = (mwl[:, pp] * cgam[pp])[None, :]
            sm_h[pp, :, MW:MW + BL] = mwl[:, pp][None, :]
            sm_h[pp, :, MWM1:MWM1 + BL] = (1.0 - mwl[:, pp])[None, :]
        per_core.append({
            "x": np.ascontiguousarray(xbf[cc * BL:(cc + 1) * BL]),
            "sm": sm_h,
            **shared,
        })
    return per_core


_CACHE = {}


def kernel(**inputs):
    from concourse.bass_utils import run_bass_kernel_spmd

    per_core = prepare_host_inputs(inputs)
    if "nc" not in _CACHE:
        _CACHE["nc"] = build_program()
    nc = _CACHE["nc"]
    res = run_bass_kernel_spmd(nc, per_core, list(range(N_CORES)))
    outs = [res.results[c]["out"] for c in range(N_CORES)]
    return np.concatenate(outs, axis=0).astype(np.float32)


# revision 9
# speedup vs baseline: 1.4463x; 1.0429x over previous
"""Trainium2 Bass kernel for ModalityAwareDualAttention (dense_cnn).

Sharding: pure data-parallel over batch (32 -> 4 per core x 8 cores).
Per core: loop over P=3 parts; each part processes all BL=4 local batches
together (token-packed attention-value matmuls, batched SE gate).

Algebraic restructurings (exact up to fp assoc.):
  - depthwise scale/bias + 2x2-avg-pool 0.25 factor folded into Wq/Wk + biases
  - v computed transposed (vT = xd^T @ Wv^T), token-packed across the 4
    batches (384 tokens -> 3 full 128-wide PE column groups)
  - v-bias commutes through softmax (rows sum to 1); added at up-eviction
  - attention apply + bilinear 2x upsample + pa_gamma fused into two matmuls:
    up = vT^T @ (attn_n @ KT), KT = gamma * K_bilinear^T
  - s := xp + up_sb (pa_out) stored at eviction; its accum gives the SE
    global-avg-pool for free (1/384 folded into fc1 weights)
  - gates folded:  cw2 = mw + mw*ca_gamma*cw,  and since cw1 - cw2 = 1 - mw,
    final = cw2*s + (1-mw)*xp  -- one fused op per tile at blend time

Pipelining: vT is computed in four 512-column passes; the up/s/fc1 work for
the finished quarter is interleaved behind the next quarter's vT matmuls so
the PE never idles waiting for PSUM evictions.  The final blend of part p is
interleaved with part p+1's x loads/pooling/projections.
Matmuls and main streams bf16 (fp32 PSUM accumulation); x and out are
carried bf16 end-to-end (tolerance 2e-2 >> bf16 rounding).
"""

import numpy as np
import ml_dtypes

import concourse.bass as bass
import concourse.tile as tile
import concourse.mybir as mybir

F32 = mybir.dt.float32
BF16 = mybir.dt.bfloat16
AF = mybir.ActivationFunctionType
ALU = mybir.AluOpType

N_CORES = 8
B, C, H, W, P = 32, 2048, 48, 24, 3
BL = B // N_CORES          # 4 local batches per core
IC = 128                   # q/k inter channels
C4 = 512                   # SE bottleneck
PH = H // P                # 16
HD, WD = PH // 2, W // 2   # 8, 12
N = HD * WD                # 96 attention tokens per batch
HWP = PH * W               # 384 spatial positions per part
KC = C // 128              # 16 channel tiles
TOK = BL * N               # 384 packed tokens per part
NG = TOK // 128            # 3 token M-groups
NCH = 4                    # vT column passes
CHW = C // NCH             # 512 columns per pass

# smalls column map
QB, KB_, B1, VBG, B2, MWC, MW, MWM1 = 0, 1, 2, 6, 22, 38, 42, 46
NS = 50


def _up_matrix(n):
    """[2n, n] bilinear x2 upsample (align_corners=False, edge clamp)."""
    M = np.zeros((2 * n, n), np.float64)
    for o in range(2 * n):
        src = (o + 0.5) / 2.0 - 0.5
        i0 = int(np.floor(src))
        f = src - i0
        M[o, min(max(i0, 0), n - 1)] += 1.0 - f
        M[o, min(max(i0 + 1, 0), n - 1)] += f
    return M


def k_bilinear():
    """[384, 96] upsample matrix: flat(16,24) <- flat(8,12)."""
    return np.kron(_up_matrix(HD), _up_matrix(WD))


def split_excess_waits(nc, max_waits=1):
    """This walrus build rejects multi-sem-wait instructions on some opcodes;
    hoist extra waits onto preceding same-engine no-ops."""
    for f in nc.m.functions:
        for bb in f.blocks:
            insts = bb.instructions
            i = 0
            while i < len(insts):
                ins = insts[i]
                si = ins.sync_info
                if si is not None and si.on_wait and len(si.on_wait) > max_waits:
                    waits = list(si.on_wait)
                    extra, keep = waits[:-max_waits], waits[-max_waits:]
                    nops = []
                    for s in range(0, len(extra), max_waits):
                        nops.append(mybir.InstNoOp(
                            name=nc.get_next_instruction_name(),
                            engine=ins.engine, ins=[], outs=[],
                            sync_info=mybir.SyncInfo(
                                on_wait=extra[s:s + max_waits], on_update=[]),
                        ))
                    ins.sync_info = mybir.SyncInfo(
                        on_wait=keep, on_update=list(si.on_update or []))
                    insts[i:i] = nops
                    i += len(nops)
                i += 1


def build_program(split_waits=True):
    from contextlib import ExitStack
    nc = bass.Bass()

    x = nc.dram_tensor("x", [BL, C, H, W], BF16, kind="ExternalInput")
    wq = nc.dram_tensor("wq", [P, 128, KC * IC], BF16, kind="ExternalInput")
    wk = nc.dram_tensor("wk", [P, 128, KC * IC], BF16, kind="ExternalInput")
    wv = nc.dram_tensor("wv", [P, 128, KC, C], BF16, kind="ExternalInput")
    kt = nc.dram_tensor("kt", [P, N, HWP], BF16, kind="ExternalInput")
    fc1 = nc.dram_tensor("fc1", [P, 4, 128, KC * 128], BF16,
                         kind="ExternalInput")
    fc2 = nc.dram_tensor("fc2", [P, 4, 128, 4 * C4], BF16,
                         kind="ExternalInput")
    sm = nc.dram_tensor("sm", [P, 128, NS], F32, kind="ExternalInput")
    out = nc.dram_tensor("out", [BL, C, H, W], BF16, kind="ExternalOutput")

    # x[b, (kc q), (p h), w] -> [b, q, kc, p, (h w)]
    xr = x.ap().rearrange("b (kc q) (p h) w -> b q kc p (h w)",
                          kc=KC, q=128, p=P)
    # out[b, (kg i q), (p h), w] -> [b, kg, q, i, p, (h w)]
    orr = out.ap().rearrange("b (kg i q) (p h) w -> b kg q i p (h w)",
                             kg=4, i=4, q=128, p=P)
    # wv[p, q, kc, c] pairs of kc per DMA
    wvr = wv.ap().rearrange("p q (k2 two) c -> p q k2 two c", two=2)

    with ExitStack() as ctx:
        tc = ctx.enter_context(tile.TileContext(nc))
        pool = lambda name, bufs, **kw: ctx.enter_context(
            tc.tile_pool(name=name, bufs=bufs, **kw))
        xp_pool = pool("xp", BL)
        xd_pool = pool("xd", 1)
        t1_pool = pool("t1", 2)
        wq_pool = pool("wqp", 2)
        wk_pool = pool("wkp", 2)
        wv_pool = pool("wvp", 3)
        kt_pool = pool("ktp", 2)
        sm_pool = pool("smp", 2)
        fc1_pool = pool("fc1p", 4)
        fc2_pool = pool("fc2p", 2)
        qk_pool = pool("qk", 4)
        at_pool = pool("at", 4)
        ss_pool = pool("ss", 8)
        g_pool = pool("gg", 5)
        vt_pool = pool("vt", NG)
        vtb_pool = pool("vtb", NG)
        s_pool = pool("sp", BL)
        ga_pool = pool("ga", 2)
        gap_pool = pool("gap", 2)
        h1_pool = pool("h1", 8)
        cws_pool = pool("cws", 4)
        cw_pool = pool("cw", 2)
        fin_pool = pool("fin", 2)
        ps_sm = pool("ps_sm", 4, space="PSUM")
        ps_vt = pool("ps_vt", NG, space="PSUM")
        ps_h = pool("ps_h", 1, space="PSUM")

        # deferred blend state from previous part
        prev = None

        def emit_blend_batch(st, b):
            """Blend batch b of a finished part: out = cw2*s + (1-mw)*xp."""
            p0, xp0, s0, cw0 = st
            for kg in range(4):
                fin = fin_pool.tile([128, 4, HWP], BF16, tag="fin",
                                    name="fin")
                for i in range(4):
                    kc = kg * 4 + i
                    nc.vector.scalar_tensor_tensor(
                        fin[:, i, :], s0[b][:, kc, :],
                        cw0[:, kc, b:b + 1], xp0[b][:, kc, :],
                        ALU.mult, ALU.add)
                nc.scalar.dma_start(orr[b, kg, :, :, p0], fin[:])

        for p in range(P):
            # ---------- per-part weight DMAs ----------
            wq_t = wq_pool.tile([128, KC * IC], BF16, tag="wq", name="wq")
            nc.sync.dma_start(wq_t[:], wq.ap()[p])
            wk_t = wk_pool.tile([128, KC * IC], BF16, tag="wk", name="wk")
            nc.sync.dma_start(wk_t[:], wk.ap()[p])
            kt_t = kt_pool.tile([N, HWP], BF16, tag="kt", name="kt")
            nc.sync.dma_start(kt_t[:], kt.ap()[p])
            sm_t = sm_pool.tile([128, NS], F32, tag="sm", name="sm")
            nc.sync.dma_start(sm_t[:], sm.ap()[p])

            # ---------- interleaved: blend(prev) | load+pool+qk(p) -------
            xp_t, qs_t, ks_t = [], [], []
            xd_t = xd_pool.tile([128, KC, TOK], BF16, tag="xd", name="xd")
            for b in range(BL):
                if prev is not None:
                    emit_blend_batch(prev, b)
                t = xp_pool.tile([128, KC, HWP], BF16, tag="xp", name="xp")
                nc.sync.dma_start(t[:], xr[b, :, :, p])
                xp_t.append(t)
                xv = t[:].rearrange("q kc (h w) -> q kc h w", h=PH)
                for k4 in range(0, KC, 4):
                    t1 = t1_pool.tile([128, 4, HD, W], BF16, tag="t1",
                                      name="t1")
                    nc.vector.tensor_tensor(
                        t1[:], xv[:, k4:k4 + 4, 0:PH:2, :],
                        xv[:, k4:k4 + 4, 1:PH:2, :], ALU.add)
                    t1v = t1[:].rearrange("q c h (w two) -> q c h w two",
                                          two=2)
                    nc.vector.scalar_tensor_tensor(
                        xd_t[:, k4:k4 + 4, b * N:(b + 1) * N].rearrange(
                            "q c (h w) -> q c h w", h=HD),
                        t1v[:, :, :, :, 0], 1.0, t1v[:, :, :, :, 1],
                        ALU.mult, ALU.add)
                # q/k per pair once both batches pooled
                if b % 2 == 1:
                    pr = b // 2
                    q_ps = ps_sm.tile([IC, 2 * N], F32, tag="ps", name="qps")
                    for kc in range(KC):
                        nc.tensor.matmul(
                            q_ps[:], wq_t[:, kc * IC:(kc + 1) * IC],
                            xd_t[:, kc, pr * 2 * N:(pr + 1) * 2 * N],
                            start=(kc == 0), stop=(kc == KC - 1))
                    qs = qk_pool.tile([IC, 2 * N], BF16, tag="qk", name="qs")
                    nc.scalar.activation(qs[:], q_ps[:], AF.Identity,
                                         bias=sm_t[:, QB:QB + 1])
                    qs_t.append(qs)
                    k_ps = ps_sm.tile([IC, 2 * N], F32, tag="ps", name="kps")
                    for kc in range(KC):
                        nc.tensor.matmul(
                            k_ps[:], wk_t[:, kc * IC:(kc + 1) * IC],
                            xd_t[:, kc, pr * 2 * N:(pr + 1) * 2 * N],
                            start=(kc == 0), stop=(kc == KC - 1))
                    ks = qk_pool.tile([IC, 2 * N], BF16, tag="qk", name="ks")
                    nc.scalar.activation(ks[:], k_ps[:], AF.Identity,
                                         bias=sm_t[:, KB_:KB_ + 1])
                    ks_t.append(ks)
            # fc1/fc2 streamed chunks (used only at SE time, loaded early)
            fc1_c = []
            for m in range(4):
                t = fc1_pool.tile([128, KC * 128], BF16, tag="fc1",
                                  name="fc1")
                nc.sync.dma_start(t[:], fc1.ap()[p, m])
                fc1_c.append(t)

            # ---------- vT in four 512-col passes; attention interleaved
            # into pass 0; up/s/fc1 for quarter q interleaved behind the
            # pass q+1 matmuls ----------
            vt_t = [vt_pool.tile([128, C], BF16, tag="vt", name="vt")
                    for _ in range(NG)]
            vtb_t = [None] + [vtb_pool.tile([N, C], BF16, tag="vtb",
                                            name="vtb")
                              for _ in range(BL - 1)]
            attn_n = [None] * BL
            g_sb = [None] * BL
            s_t = [s_pool.tile([128, KC, HWP], BF16, tag="sp", name="s")
                   for _ in range(BL)]
            ga_t = ga_pool.tile([128, KC, BL], F32, tag="ga", name="ga")
            gap_t = gap_pool.tile([128, KC, BL], BF16, tag="gap", name="gap")
            h_all = ps_h.tile([128, 16], F32, tag="psh", name="hall")

            def up_quarter(ch):
                """up matmuls + s eviction + fc1 for kc in quarter ch."""
                for kc in range(4 * ch, 4 * ch + 4):
                    for j in range(BL):
                        lhs = (vt_t[0] if j == 0 else vtb_t[j])
                        lhs = lhs[0:N, kc * 128:(kc + 1) * 128]
                        up_ps = ps_sm.tile([128, HWP], F32, tag="ps",
                                           name="ups")
                        nc.tensor.matmul(up_ps[:], lhs, g_sb[j][:],
                                         start=True, stop=True)
                        nc.vector.scalar_tensor_tensor(
                            s_t[j][:, kc, :], up_ps[:],
                            sm_t[:, VBG + kc:VBG + kc + 1],
                            xp_t[j][:, kc, :],
                            ALU.add, ALU.add,
                            accum_out=ga_t[:, kc, j:j + 1])
                    nc.gpsimd.tensor_copy(gap_t[:, kc, :], ga_t[:, kc, :])
                    for m in range(4):
                        nc.tensor.matmul(
                            h_all[:, m * 4:(m + 1) * 4],
                            fc1_c[m][:, kc * 128:(kc + 1) * 128],
                            gap_t[:, kc, :],
                            start=(kc == 0), stop=(kc == KC - 1))

            for ch in range(NCH):
                cl = ch * CHW
                vt_ps = [ps_vt.tile([128, CHW], F32, tag="psv", name="vps")
                         for _ in range(NG)]
                for k2 in range(KC // 2):
                    wv_t = wv_pool.tile([128, 2, CHW], BF16, tag="wv",
                                        name="wv")
                    nc.sync.dma_start(wv_t[:], wvr[p, :, k2, :, cl:cl + CHW])
                    for i2 in range(2):
                        kc = 2 * k2 + i2
                        for gi in range(NG):
                            nc.tensor.matmul(
                                vt_ps[gi][:],
                                xd_t[:, kc, gi * 128:(gi + 1) * 128],
                                wv_t[:, i2, :],
                                start=(kc == 0), stop=(kc == KC - 1))
                    # interleave attention (softmax without max-shift:
                    # |energy| ~ 1e-3, exp cannot overflow)
                    if ch == 0:
                        j, ph = k2 // 2, k2 % 2
                        if ph == 0:
                            pr, jo = j // 2, (j % 2) * N
                            e_ps = ps_sm.tile([N, N], F32, tag="ps",
                                              name="eps")
                            nc.tensor.matmul(
                                e_ps[:], qs_t[pr][:, jo:jo + N],
                                ks_t[pr][:, jo:jo + N],
                                start=True, stop=True)
                            a_e = at_pool.tile([N, N], BF16, tag="at",
                                               name="ae")
                            s_sum = ss_pool.tile([N, 1], F32, tag="ss",
                                                 name="ssum")
                            nc.scalar.activation(a_e[:], e_ps[:], AF.Exp,
                                                 accum_out=s_sum[:])
                            r_sum = ss_pool.tile([N, 1], F32, tag="ss",
                                                 name="rsum")
                            nc.vector.reciprocal(r_sum[:], s_sum[:])
                            a_n = at_pool.tile([N, N], BF16, tag="at",
                                               name="an")
                            nc.vector.tensor_scalar(a_n[:], a_e[:],
                                                    r_sum[:], None,
                                                    ALU.mult)
                            attn_n[j] = a_n
                        else:
                            g_ps = ps_sm.tile([N, HWP], F32, tag="ps",
                                              name="gps")
                            nc.tensor.matmul(g_ps[:], attn_n[j][:],
                                             kt_t[:], start=True, stop=True)
                            gt = g_pool.tile([N, HWP], BF16, tag="g",
                                             name="g")
                            nc.scalar.activation(gt[:], g_ps[:], AF.Copy)
                            g_sb[j] = gt
                for gi in range(NG):
                    nc.scalar.activation(vt_t[gi][:, cl:cl + CHW],
                                         vt_ps[gi][:], AF.Copy)
                # token re-base: batch j's 96 token rows contiguous from 0
                cs = slice(cl, cl + CHW)
                nc.gpsimd.dma_start(vtb_t[1][0:32, cs], vt_t[0][96:128, cs])
                nc.gpsimd.dma_start(vtb_t[1][32:96, cs], vt_t[1][0:64, cs])
                nc.gpsimd.dma_start(vtb_t[2][0:64, cs], vt_t[1][64:128, cs])
                nc.gpsimd.dma_start(vtb_t[2][64:96, cs], vt_t[2][0:32, cs])
                nc.gpsimd.dma_start(vtb_t[3][0:96, cs], vt_t[2][32:128, cs])
                up_quarter(ch)

            # xp scaled by (1-mw) for the final blend (whole batch at once)
            for j in range(BL):
                nc.scalar.activation(
                    xp_t[j][:].rearrange("q kc f -> q (kc f)"),
                    xp_t[j][:].rearrange("q kc f -> q (kc f)"), AF.Copy,
                    scale=sm_t[:, MWM1 + j:MWM1 + j + 1])

            # ---------- SE gate tail ----------
            h1_t = []
            for m in range(4):
                hb = h1_pool.tile([128, BL], BF16, tag="h1", name="h1")
                nc.scalar.activation(hb[:], h_all[:, m * 4:(m + 1) * 4],
                                     AF.Relu, bias=sm_t[:, B1 + m:B1 + m + 1])
                h1_t.append(hb)
            fc2_c = []
            for kg in range(4):
                t = fc2_pool.tile([128, 4 * C4], BF16, tag="fc2",
                                  name="fc2")
                nc.sync.dma_start(t[:], fc2.ap()[p, kg])
                fc2_c.append(t)
            cw2_t = cw_pool.tile([128, KC, BL], F32, tag="cw", name="cw2")
            for kc in range(KC):
                c_ps = ps_sm.tile([128, BL], F32, tag="ps", name="cps")
                for m in range(4):
                    nc.tensor.matmul(
                        c_ps[:],
                        fc2_c[kc // 4][:, (kc % 4) * C4 + m * 128:
                                       (kc % 4) * C4 + (m + 1) * 128],
                        h1_t[m][:], start=(m == 0), stop=(m == 3))
                cwg = cws_pool.tile([128, BL], F32, tag="cws", name="cwg")
                nc.scalar.activation(cwg[:], c_ps[:], AF.Sigmoid,
                                     bias=sm_t[:, B2 + kc:B2 + kc + 1])
                # cw2 = mw + mwc*cw
                tmp = cws_pool.tile([128, BL], F32, tag="cws", name="tmp")
                nc.vector.tensor_tensor(tmp[:], cwg[:],
                                        sm_t[:, MWC:MWC + BL], ALU.mult)
                nc.vector.tensor_tensor(cw2_t[:, kc, :], tmp[:],
                                        sm_t[:, MW:MW + BL], ALU.add)

            prev = (p, xp_t, s_t, cw2_t)

        # final part's blend
        for b in range(BL):
            emit_blend_batch(prev, b)

    if split_waits:
        split_excess_waits(nc)
    return nc


# ---------------------------------------------------------------------------
# Host side
# ---------------------------------------------------------------------------

def _sigmoid(v):
    return 1.0 / (1.0 + np.exp(-v))


def _bf(a):
    return np.ascontiguousarray(a.astype(ml_dtypes.bfloat16))


def prepare_host_inputs(inputs):
    """Fold/transpose weights; returns per-core input dicts."""
    g = {k: np.asarray(v) for k, v in inputs.items()}
    x = np.asarray(g["x"], dtype=np.float32)

    # modality gate on host (tiny): mw [B, P]
    mf = g["modality"].astype(np.float64)[:, None]
    g1 = np.maximum(mf @ g["gate_w1"].astype(np.float64).T
                    + g["gate_b1"].astype(np.float64), 0.0)
    mw = _sigmoid(g1 @ g["gate_w2"].astype(np.float64).T
                  + g["gate_b2"].astype(np.float64))      # [B, P]

    paq = g["pa_q_w"].astype(np.float64)    # [P, IC, C]
    pak = g["pa_k_w"].astype(np.float64)
    pav = g["pa_v_w"].astype(np.float64)    # [P, C, C]
    dwq_w = g["pa_dw_q_w"].astype(np.float64)   # [P, C]
    dwq_b = g["pa_dw_q_b"].astype(np.float64)
    dwk_w = g["pa_dw_k_w"].astype(np.float64)
    dwk_b = g["pa_dw_k_b"].astype(np.float64)
    gam = g["pa_gamma"].astype(np.float64)      # [P]
    cgam = g["ca_gamma"].astype(np.float64)

    def chunkT(a, n128, inner):
        # [C_outer, inner] -> [128, n128 * inner], partition-major
        return a.reshape(n128, 128, inner).transpose(1, 0, 2).reshape(
            128, n128 * inner)

    wq_h = np.stack([chunkT((paq[pp] * dwq_w[pp][None, :] * 0.25).T, KC, IC)
                     for pp in range(P)])
    wk_h = np.stack([chunkT((pak[pp] * dwk_w[pp][None, :] * 0.25).T, KC, IC)
                     for pp in range(P)])
    qb_h = np.stack([g["pa_q_b"][pp] + paq[pp] @ dwq_b[pp] for pp in range(P)])
    kb_h = np.stack([g["pa_k_b"][pp] + pak[pp] @ dwk_b[pp] for pp in range(P)])
    wv_h = np.stack([chunkT(0.25 * pav[pp].T, KC, C).reshape(128, KC, C)
                     for pp in range(P)])
    vbg_h = np.stack([(gam[pp] * g["pa_v_b"][pp].astype(np.float64))
                      .reshape(KC, 128).T for pp in range(P)])  # [P,128,16]

    kb_mat = k_bilinear()                     # [384, 96]
    kt_h = np.stack([gam[pp] * kb_mat.T for pp in range(P)])    # [P,96,384]

    fc1w = g["ca_fc1_w"].astype(np.float64)    # [P, C4, C]
    fc2w = g["ca_fc2_w"].astype(np.float64)    # [P, C, C4]
    # fc1': (fc1_w/384).T [C, C4] -> [m, q(c chunk part.), kc, j] chunks
    fc1_h = np.stack([
        (fc1w[pp] / HWP).T.reshape(KC, 128, 4, 128)
        .transpose(2, 1, 0, 3).reshape(4, 128, KC * 128)
        for pp in range(P)])
    # fc2': fc2_w.T [C4, C] -> [kcg, q(c4 chunk part.), ci, m, j]
    fc2_h = np.stack([
        fc2w[pp].T.reshape(4, 128, 4, 4, 128)
        .transpose(2, 1, 3, 0, 4).reshape(4, 128, 4 * C4)
        for pp in range(P)])
    b1_h = np.stack([g["ca_fc1_b"][pp].reshape(4, 128).T for pp in range(P)])
    b2_h = np.stack([g["ca_fc2_b"][pp].reshape(KC, 128).T for pp in range(P)])

    shared = {
        "wq": _bf(wq_h), "wk": _bf(wk_h), "wv": _bf(wv_h),
        "kt": _bf(kt_h), "fc1": _bf(fc1_h.reshape(P, 4, 128, KC * 128)),
        "fc2": _bf(fc2_h.reshape(P, 4, 128, 4 * C4)),
    }
    xbf = x.astype(ml_dtypes.bfloat16)
    per_core = []
    for cc in range(N_CORES):
        mwl = mw[cc * BL:(cc + 1) * BL]       # [BL, P]
        sm_h = np.zeros((P, 128, NS), np.float32)
        for pp in range(P):
            sm_h[pp, :, QB] = qb_h[pp]
            sm_h[pp, :, KB_] = kb_h[pp]
            sm_h[pp, :, B1:B1 + 4] = b1_h[pp]
            sm_h[pp, :, VBG:VBG + KC] = vbg_h[pp]
            sm_h[pp, :, B2:B2 + KC] = b2_h[pp]
            sm_h[pp, :, MWC:MWC + BL] = (mwl[:, pp] * cgam[pp])[None, :]
            sm_h[pp, :, MW:MW + BL] = mwl[:, pp][None, :]
            sm_h[pp, :, MWM1:MWM1 + BL] = (1.0 - mwl[:, pp])[None, :]
        per_core.append({
            "x": np.ascontiguousarray(xbf[cc * BL:(cc + 1) * BL]),
            "sm": sm_h,
            **shared,
        })
    return per_core


_CACHE = {}


def kernel(**inputs):
    from concourse.bass_utils import run_bass_kernel_spmd

    per_core = prepare_host_inputs(inputs)
    if "nc" not in _CACHE:
        _CACHE["nc"] = build_program()
    nc = _CACHE["nc"]
    res = run_bass_kernel_spmd(nc, per_core, list(range(N_CORES)))
    outs = [res.results[c]["out"] for c in range(N_CORES)]
    return np.concatenate(outs, axis=0).astype(np.float32)
